# revision 1
# baseline (speedup 1.0000x reference)
"""MultiHeadAttention TRN2 Bass kernel.

Problem: B=4, S=2048, D=768, H=12 heads (DK=64).
Sharding: 8 cores = (batch b in 0..3) x (head-half in 0..1); each core
computes 6 heads of one batch element end-to-end (tensor-parallel over
heads within a batch). Host pre-transposes activations to [D, S] (and
casts to bf16 in the default fast path), slices projection weights per
head-half, and sums the two partial outputs per batch (+ bv@Wo + bo
correction, exact because softmax rows sum to 1).

On-core math:
  qh^T[384, S]: lhsT=Wq tile [Din,dout], rhs=q^T tile [Din,s] (+bq in drain)
  kh^T likewise; vh natural [S, 390] via lhsT=v^T tile, rhs=Wv:
    vh_aug[s, 65j..65j+64] = [m(s)*vh_head_j(s, :), m(s)]  (mask fold)
  S^T[k, q] = kh_head^T.T @ qh_head^T  (contraction d=64)
  P^T = exp(S^T * 0.125)               (ACT, fused scale, no max-sub)
  ctx_aug^T[0:65, q] += vh_aug_j[kc].T @ P^T[kc]  over k-chunks
    rows 0..63 = unnormalized ctx^T, row 64 = softmax denominator
  rs = approx-recip(denom); bcast via ones[1,64] K=1 f32r matmul;
  cn = ctx^T * rs   (drains deferred one (h,qb) iteration so the PE
                     pipeline never waits on the reciprocal chain)
  out[q, 768] = sum_dt cn[dt].T @ Wo tiles  (per 128-q chunk)
"""

import os
import sys
import types
from contextlib import ExitStack

import ml_dtypes
import numpy as np

import concourse.bacc as bacc
import concourse.bass as bass
import concourse.mybir as mybir
import concourse.tile as tile
from concourse import bass_utils
from concourse.bass import ts, ds

F32 = mybir.dt.float32
F32R = mybir.dt.float32r
BF16 = mybir.dt.bfloat16

D = 768        # model dim
DH = 384       # per-core head dim (6 heads x 64)
HPC = 6        # heads per core
VW = HPC * 65  # vh_aug free width (390)
QB = 1024      # q-block width in attention phase


def build_nc(S=2048, bf16=True):
    nc = bacc.Bacc("TRN2", target_bir_lowering=False, debug=False)

    MMD = BF16 if bf16 else F32R    # matmul operand dtype
    CW = min(1024, S)               # phase-1 s-chunk width
    QBW = min(512, S)               # attention q-block width
    NKT = S // 128                  # 128-wide s/k tiles
    NQB = S // QBW                  # q blocks
    NSC = S // 512                  # 512-wide s chunks

    qT = nc.dram_tensor("qT", [D, S], MMD, kind="ExternalInput").ap()
    kT = nc.dram_tensor("kT", [D, S], MMD, kind="ExternalInput").ap()
    vT = nc.dram_tensor("vT", [D, S], MMD, kind="ExternalInput").ap()
    wq = nc.dram_tensor("wq", [D, DH], MMD, kind="ExternalInput").ap()
    wk = nc.dram_tensor("wk", [D, DH], MMD, kind="ExternalInput").ap()
    wv = nc.dram_tensor("wv", [D, DH], MMD, kind="ExternalInput").ap()
    wo = nc.dram_tensor("wo", [DH, D], MMD, kind="ExternalInput").ap()
    bq = nc.dram_tensor("bq", [DH, 1], F32, kind="ExternalInput").ap()
    bk = nc.dram_tensor("bk", [DH, 1], F32, kind="ExternalInput").ap()
    mv = nc.dram_tensor("mv", [S, 1], F32, kind="ExternalInput").ap()
    out = nc.dram_tensor("out", [S, D], F32, kind="ExternalOutput").ap()

    with tile.TileContext(nc) as tc, ExitStack() as ctx:
        P = 128
        wpool = ctx.enter_context(tc.tile_pool(name="w", bufs=1))
        xin = ctx.enter_context(tc.tile_pool(name="xin", bufs=8))
        persist = ctx.enter_context(tc.tile_pool(name="persist", bufs=1))
        ppool = ctx.enter_context(tc.tile_pool(name="p", bufs=3))
        small = ctx.enter_context(tc.tile_pool(name="small", bufs=2))
        outp = ctx.enter_context(tc.tile_pool(name="outp", bufs=2))
        psA = ctx.enter_context(tc.tile_pool(name="psA", bufs=2, space="PSUM"))
        psB = ctx.enter_context(tc.tile_pool(name="psB", bufs=4, space="PSUM"))

        # ---- constants / small tensors ----
        wq_sb = [wpool.tile([P, DH], MMD, name=f"wq{c}", tag=f"wq{c}") for c in range(6)]
        wk_sb = [wpool.tile([P, DH], MMD, name=f"wk{c}", tag=f"wk{c}") for c in range(6)]
        wv_sb = [wpool.tile([P, DH], MMD, name=f"wv{c}", tag=f"wv{c}") for c in range(6)]
        wo_sb = [wpool.tile([P, D], MMD, name=f"wo{c}", tag=f"wo{c}") for c in range(3)]
        for c in range(6):
            (nc.sync if c % 2 == 0 else nc.gpsimd).dma_start(
                wk_sb[c][:], wk[ts(c, P), :]
            )
        bq_sb = [wpool.tile([P, 1], F32, name=f"bq{t}", tag=f"bq{t}") for t in range(3)]
        bk_sb = [wpool.tile([P, 1], F32, name=f"bk{t}", tag=f"bk{t}") for t in range(3)]
        for t in range(3):
            nc.sync.dma_start(bq_sb[t][:], bq[ts(t, P), :])
            nc.sync.dma_start(bk_sb[t][:], bk[ts(t, P), :])
        mv_sb = [wpool.tile([P, 1], F32, name=f"mv{st}", tag=f"mv{st}") for st in range(NKT)]
        for st in range(NKT):
            nc.sync.dma_start(mv_sb[st][:], mv[ts(st, P), :])
        ones6 = wpool.tile([P, HPC], F32, tag="ones6")
        nc.vector.memset(ones6[:], 1.0)
        ones64f = wpool.tile([1, 64], F32, tag="ones64f")
        nc.vector.memset(ones64f[:], 1.0)
        ones64 = wpool.tile([1, 64], F32R, tag="ones64")
        nc.vector.tensor_copy(ones64[:], ones64f[:])

        # ---- persistent activations ----
        khT = [persist.tile([P, S], MMD, name=f"khT{t}", tag=f"khT{t}") for t in range(3)]
        qhT = [persist.tile([P, S], MMD, name=f"qhT{t}", tag=f"qhT{t}") for t in range(3)]
        vh = [persist.tile([P, VW], MMD, name=f"vh{st}", tag=f"vh{st}") for st in range(NKT)]
        cn = [persist.tile([P, S], MMD, name=f"cn{t}", tag=f"cn{t}") for t in range(3)]

        # ---- phase 1a/1b: projections (q-proj of the second q-block is
        # deferred into the attention stream as PE filler work) ----
        def proj_chunk(xdram, wsb, bsb, dst, sc, pools=("psA", "psA", "psA")):
            xt = [xin.tile([P, CW], MMD, name="xin", tag="xin") for c in range(6)]
            for c in range(6):
                (nc.sync if c % 2 == 0 else nc.gpsimd).dma_start(
                    xt[c][:], xdram[ts(c, P), ts(sc, CW)]
                )
            for dt in range(3):
                ps = psA.tile([P, CW], F32, name="psA", tag="psA")
                for u in range(CW // 512):
                    for c in range(6):
                        nc.tensor.matmul(
                            ps[:, ts(u, 512)],
                            lhsT=wsb[c][:, ts(dt, P)],
                            rhs=xt[c][:, ts(u, 512)],
                            start=(c == 0),
                            stop=(c == 5),
                        )
                nc.vector.tensor_scalar_add(
                    out=dst[dt][:, ts(sc, CW)], in0=ps[:],
                    scalar1=bsb[dt][:],
                )

        # k-projection
        for sc in range(S // CW):
            proj_chunk(kT, wk_sb, bk_sb, khT, sc)

        # v-projection
        for c in range(6):
            nc.sync.dma_start(wv_sb[c][:], wv[ts(c, P), :])
        for stb in range(0, NKT, CW // 128):
            vt = [xin.tile([P, CW], MMD, name="xin", tag="xin") for c in range(6)]
            for c in range(6):
                nc.sync.dma_start(vt[c][:], vT[ts(c, P), ds(stb * 128, CW)])
            for sj in range(CW // 128):
                st = stb + sj
                ps = psA.tile([P, CW], F32, name="psA", tag="psA")
                for c in range(6):
                    nc.tensor.matmul(
                        ps[:, :DH],
                        lhsT=vt[c][:, ts(sj, P)],
                        rhs=wv_sb[c][:],
                        start=(c == 0),
                        stop=(c == 5),
                    )
                vh3 = vh[st].rearrange("p (h c) -> p h c", c=65)
                nc.vector.tensor_scalar_mul(
                    out=vh3[:, :, 0:64],
                    in0=ps[:, :DH].rearrange("p (h c) -> p h c", c=64),
                    scalar1=mv_sb[st][:],
                )
                nc.vector.tensor_scalar_mul(
                    out=vh3[:, :, 64:65],
                    in0=ones6[:].rearrange("p (h c) -> p h c", c=1),
                    scalar1=mv_sb[st][:],
                )

        # q-projection: first q-block now, rest deferred into phase 2
        for c in range(6):
            nc.sync.dma_start(wq_sb[c][:], wq[ts(c, P), :])
        proj_chunk(qT, wq_sb, bq_sb, qhT, 0)
        for c in range(3):
            nc.sync.dma_start(wo_sb[c][:], wo[ts(c, P), :])
        pend_qproj = [
            (sc, dt, u)
            for sc in range(1, S // CW)
            for dt in range(3)
            for u in range(CW // 512)
        ]
        qproj_xt = {}
        for sc in range(1, S // CW):
            qproj_xt[sc] = [
                xin.tile([P, CW], MMD, name="xin", tag="xin") for c in range(6)
            ]
            for c in range(6):
                (nc.sync if c % 2 == 0 else nc.gpsimd).dma_start(
                    qproj_xt[sc][c][:], qT[ts(c, P), ts(sc, CW)]
                )

        # ---- phase 2: attention, head-pair steps ----
        # Each step handles BOTH heads of a pair for one k-chunk: the two
        # scores matmuls live in disjoint PE row groups (base partition 0
        # and 64) so they run concurrently, and share one [128,1024] PSUM
        # tile (head A in cols 0:512, head B in 512:1024) -> one exp per
        # step. Scores run 2 steps ahead of attn@V. Drains and filler work
        # (deferred q-proj, O-proj) go to psB spare slots, never stalling
        # the scores pipeline on psA.
        hq = [(pr, qb) for qb in range(NQB) for pr in range(3)]
        steps = [(pr, qb, kc) for (pr, qb) in hq for kc in range(NKT)]

        ctx_ps = {}
        st_ps = {}

        def scores(pr, qb, kc):
            ps = psA.tile([P, 1024], F32, name="psA", tag="psA")
            for hh in range(2):
                nc.tensor.matmul(
                    ps[:, ts(hh, 512)],
                    lhsT=khT[pr][64 * hh : 64 * hh + 64, ts(kc, P)],
                    rhs=qhT[pr][64 * hh : 64 * hh + 64, ts(qb, QBW)],
                    start=True,
                    stop=True,
                )
            st_ps[(pr, qb, kc)] = ps

        def attnv(pr, qb, kc, pt):
            for hh in range(2):
                h = 2 * pr + hh
                nc.tensor.matmul(
                    ctx_ps[(h, qb)][0:65, :],
                    lhsT=vh[kc][:, ds(65 * h, 65)],
                    rhs=pt[:, ts(hh, 512)],
                    start=(kc == 0),
                    stop=(kc == NKT - 1),
                )

        def drain(h, qb):
            """Normalize + store ctx for a finished (h, qb)."""
            dt, pb = h // 2, 64 * (h % 2)
            cps = ctx_ps.pop((h, qb))
            rs = small.tile([1, QBW], F32, name="rs", tag="rs")
            with nc.allow_low_precision(reason="softmax denom recip"):
                nc.vector.reciprocal(rs[:], cps[64:65, :])
            bcs = small.tile([64, QBW], F32, name="bcs", tag="bcs")
            nc.gpsimd.partition_broadcast(bcs[:], rs[:])
            if pb == 0:
                nc.vector.tensor_tensor(
                    out=cn[dt][0:64, ts(qb, QBW)],
                    in0=cps[0:64, :],
                    in1=bcs[:],
                    op=mybir.AluOpType.mult,
                )
            else:
                tmp = small.tile([64, QBW], MMD, name="tmp", tag="tmp")
                nc.vector.tensor_tensor(
                    out=tmp[:], in0=cps[0:64, :], in1=bcs[:],
                    op=mybir.AluOpType.mult,
                )
                nc.sync.dma_start(cn[dt][64:128, ts(qb, QBW)], tmp[:])

        def oproj(qc):
            ups = psB.tile([P, 512], F32, name="psB", tag="psB")
            ups2 = psB.tile([P, 256], F32, name="psB2", tag="psB")
            for ps_, n0, nw in ((ups, 0, 512), (ups2, 512, 256)):
                for dt in range(3):
                    nc.tensor.matmul(
                        ps_[:, 0:nw],
                        lhsT=cn[dt][:, ts(qc, P)],
                        rhs=wo_sb[dt][:, ds(n0, nw)],
                        start=(dt == 0),
                        stop=(dt == 2),
                    )
            ot = outp.tile([P, D], F32, name="ot", tag="ot")
            nc.vector.tensor_copy(ot[:, 0:512], ups[:, 0:512])
            nc.vector.tensor_copy(ot[:, 512:768], ups2[:, 0:256])
            nc.sync.dma_start(out[ts(qc, P), :], ot[:])

        def qproj_sub(sc, dt, u):
            ps = psB.tile([P, 512], F32, name="psB", tag="psB")
            for c in range(6):
                nc.tensor.matmul(
                    ps[:],
                    lhsT=wq_sb[c][:, ts(dt, P)],
                    rhs=qproj_xt[sc][c][:, ts(u, 512)],
                    start=(c == 0),
                    stop=(c == 5),
                )
            nc.vector.tensor_scalar_add(
                out=qhT[dt][:, ds(sc * CW + u * 512, 512)], in0=ps[:],
                scalar1=bq_sb[dt][:],
            )
            if dt == 2 and u == CW // 512 - 1:
                qproj_xt.pop(sc)

        DEPTH = 2
        pend_drain = []
        pend_oproj = []
        for n, (pr, qb, kc) in enumerate(steps):
            if kc == 0:
                for hh in range(2):
                    ctx_ps[(2 * pr + hh, qb)] = psB.tile(
                        [P, QBW], F32, name="psB", tag="psB"
                    )[0:65, :]
            if n < DEPTH:
                scores(*steps[n])
            pt = ppool.tile([P, 1024], MMD, name="pt", tag="pt")
            nc.scalar.activation(
                pt[:], st_ps.pop((pr, qb, kc))[:],
                mybir.ActivationFunctionType.Exp, scale=0.125,
            )
            if n + DEPTH < len(steps):
                scores(*steps[n + DEPTH])
            attnv(pr, qb, kc, pt)
            if kc in (2, 5) and pend_drain:
                hd, qd = pend_drain.pop(0)
                drain(hd, qd)
                if hd == HPC - 1:
                    pend_oproj.extend(range(qd * (QBW // P), (qd + 1) * (QBW // P)))
            elif pend_oproj and kc in (9, 11, 13):
                oproj(pend_oproj.pop(0))
            elif pend_qproj and kc == 15:
                qproj_sub(*pend_qproj.pop(0))
            if kc == NKT - 1:
                pend_drain.extend([(2 * pr, qb), (2 * pr + 1, qb)])
        for hd, qd in pend_drain:
            drain(hd, qd)
            if hd == HPC - 1:
                pend_oproj.extend(range(qd * (QBW // P), (qd + 1) * (QBW // P)))
        for qc in pend_oproj:
            oproj(qc)

    nc.compile()
    return nc


_NC_CACHE = {}


def _get_nc(S, bf16=True):
    key = (S, bf16)
    if key not in _NC_CACHE:
        _NC_CACHE[key] = build_nc(S, bf16)
    return _NC_CACHE[key]


def _install_ntff_hook():
    try:
        mod = types.ModuleType("antenv.axon_hooks")
        state = {"hook": None}
        mod.set_axon_ntff_profile_hook = lambda h: state.__setitem__("hook", h)
        mod.get_axon_ntff_profile_hook = lambda: state["hook"]
        sys.modules["antenv.axon_hooks"] = mod
        from trn_agent_boot.trn_boot import _ntff_profile_via_ctypes

        mod.set_axon_ntff_profile_hook(
            _ntff_profile_via_ctypes("/opt/axon/libaxon_pjrt.so")
        )
        bass_utils.upload_artifacts = lambda tmpdir: "local://" + tmpdir
        return state["hook"] is not None
    except Exception:
        return False


def run_cores(in_maps, S=2048, bf16=True, profile=False):
    nc = _get_nc(S, bf16)
    trace = bool(profile) and _install_ntff_hook()
    res = bass_utils.run_bass_kernel_spmd(
        nc, in_maps, core_ids=list(range(len(in_maps))), trace=trace
    )
    return res


def make_in_maps(q, k, v, mask, Wq, bq, Wk, bk, Wv, Wo, bf16=True):
    B = q.shape[0]
    mmd = ml_dtypes.bfloat16 if bf16 else np.float32
    qT = np.ascontiguousarray(
        np.asarray(q, np.float32).transpose(0, 2, 1)).astype(mmd)
    kT = np.ascontiguousarray(
        np.asarray(k, np.float32).transpose(0, 2, 1)).astype(mmd)
    vT = np.ascontiguousarray(
        np.asarray(v, np.float32).transpose(0, 2, 1)).astype(mmd)
    mvec = (~np.asarray(mask).reshape(B, -1)).astype(np.float32)
    Wq, Wk, Wv, Wo = (np.asarray(a, np.float32) for a in (Wq, Wk, Wv, Wo))
    bq, bk = np.asarray(bq, np.float32), np.asarray(bk, np.float32)
    in_maps = []
    for b in range(B):
        for half in range(2):
            hs = slice(DH * half, DH * (half + 1))
            in_maps.append(
                {
                    "qT": qT[b],
                    "kT": kT[b],
                    "vT": vT[b],
                    "wq": np.ascontiguousarray(Wq[:, hs]).astype(mmd),
                    "wk": np.ascontiguousarray(Wk[:, hs]).astype(mmd),
                    "wv": np.ascontiguousarray(Wv[:, hs]).astype(mmd),
                    "wo": np.ascontiguousarray(Wo[hs, :]).astype(mmd),
                    "bq": np.ascontiguousarray(bq[hs]).reshape(DH, 1),
                    "bk": np.ascontiguousarray(bk[hs]).reshape(DH, 1),
                    "mv": np.ascontiguousarray(mvec[b]).reshape(-1, 1),
                }
            )
    return in_maps


def kernel(q, k, v, mask, Wq, bq, Wk, bk, Wv, bv, Wo, bo):
    q = np.asarray(q, np.float32)
    B, S, _ = q.shape
    bf16 = os.environ.get("BASS_PRECISE") != "1"
    in_maps = make_in_maps(q, k, v, mask, Wq, bq, Wk, bk, Wv, Wo, bf16=bf16)
    res = run_cores(
        in_maps, S=S, bf16=bf16, profile=os.environ.get("BASS_PROFILE") == "1"
    )
    if os.environ.get("BASS_PROFILE") == "1" and res.exec_time_ns is not None:
        print(f"HW exec time: {res.exec_time_ns} ns")
    cvec = (
        np.asarray(bv, np.float32) @ np.asarray(Wo, np.float32)
        + np.asarray(bo, np.float32)
    )
    out = np.empty((B, S, D), np.float32)
    for b in range(B):
        out[b] = res.results[2 * b]["out"] + res.results[2 * b + 1]["out"] + cvec
    return out



# revision 4
# speedup vs baseline: 1.8039x; 1.8039x over previous
"""MultiHeadAttention TRN2 Bass kernel.

Problem: B=4, S=2048, D=768, H=12 heads (DK=64).
Sharding: 8 cores = (batch b in 0..3) x (head-half in 0..1); each core
computes 6 heads of one batch element end-to-end (tensor-parallel over
heads within a batch). Host pre-transposes activations to [D, S] (and
casts to bf16 in the default fast path), slices projection weights per
head-half, and sums the two partial outputs per batch (+ bv@Wo + bo
correction, exact because softmax rows sum to 1).

Key compaction: the boolean mask drops ~half the key positions, so the
host gathers the unmasked keys to the front of kT/vT (padded with zeros
to a multiple of 128, SK columns total) and sets mv=1 for real keys /
0 for padding. The padded columns flow through the same mask-fold math
(vh_aug *= mv) the full kernel used, so the result is exact while the
score/exp/attn@V/k-proj/v-proj work drops to SK/S of the dense cost.

On-core math:
  qh^T[384, S]: lhsT=Wq tile [Din,dout], rhs=q^T tile [Din,s] (+bq in drain)
  kh^T likewise over SK; vh natural [SK, 390] via lhsT=v^T tile, rhs=Wv:
    vh_aug[s, 65j..65j+64] = [m(s)*vh_head_j(s, :), m(s)]  (mask fold)
  S^T[k, q] = kh_head^T.T @ qh_head^T  (contraction d=64)
  P^T = exp(S^T * 0.125)               (ACT, fused scale, no max-sub)
  ctx_aug^T[0:65, q] += vh_aug_j[kc].T @ P^T[kc]  over k-chunks
    rows 0..63 = unnormalized ctx^T, row 64 = softmax denominator
  rs = recip-approx(denom); bcast via gpsimd partition_broadcast;
  cn = ctx^T * rs   (drains deferred one (h,qb) iteration so the PE
                     pipeline never waits on the reciprocal chain)
  out[q, 768] = sum_dt cn[dt].T @ Wo tiles  (per 128-q chunk, split in
  two PSUM-bank-sized halves so transient PSUM stays at 1 bank)
"""

import os
import sys
import types
from contextlib import ExitStack

import ml_dtypes
import numpy as np

import concourse.bacc as bacc
import concourse.bass as bass
import concourse.mybir as mybir
import concourse.tile as tile
from concourse import bass_utils
from concourse.bass import ts, ds

F32 = mybir.dt.float32
F32R = mybir.dt.float32r
BF16 = mybir.dt.bfloat16

D = 768        # model dim
DH = 384       # per-core head dim (6 heads x 64)
HPC = 6        # heads per core
VW = HPC * 65  # vh_aug free width (390)


def _chunks(total, w=512):
    out = []
    off = 0
    while off < total:
        cw = min(w, total - off)
        out.append((off, cw))
        off += cw
    return out


def build_nc(S=2048, SK=1152, bf16=True):
    nc = bacc.Bacc("TRN2", target_bir_lowering=False, debug=False)

    MMD = BF16 if bf16 else F32R    # matmul operand dtype
    QBW = min(512, S)               # attention q-block width
    NKT = SK // 128                 # 128-wide k tiles
    NQB = S // QBW                  # q blocks
    KCH = _chunks(SK)               # k/v projection chunks (<=512 wide)
    QCH = _chunks(S)                # q projection chunks (512 wide)

    qT = nc.dram_tensor("qT", [D, S], MMD, kind="ExternalInput").ap()
    kT = nc.dram_tensor("kT", [D, SK], MMD, kind="ExternalInput").ap()
    vT = nc.dram_tensor("vT", [D, SK], MMD, kind="ExternalInput").ap()
    wq = nc.dram_tensor("wq", [D, DH], MMD, kind="ExternalInput").ap()
    wk = nc.dram_tensor("wk", [D, DH], MMD, kind="ExternalInput").ap()
    wv = nc.dram_tensor("wv", [D, DH], MMD, kind="ExternalInput").ap()
    wo = nc.dram_tensor("wo", [DH, D], MMD, kind="ExternalInput").ap()
    bq = nc.dram_tensor("bq", [DH, 1], F32, kind="ExternalInput").ap()
    bk = nc.dram_tensor("bk", [DH, 1], F32, kind="ExternalInput").ap()
    mv = nc.dram_tensor("mv", [SK, 1], F32, kind="ExternalInput").ap()
    out = nc.dram_tensor("out", [S, D], F32, kind="ExternalOutput").ap()

    with tile.TileContext(nc) as tc, ExitStack() as ctx:
        P = 128
        wpool = ctx.enter_context(tc.tile_pool(name="w", bufs=1))
        xin = ctx.enter_context(tc.tile_pool(name="xin", bufs=8))
        qdef = ctx.enter_context(tc.tile_pool(name="qdef", bufs=18))
        persist = ctx.enter_context(tc.tile_pool(name="persist", bufs=1))
        ppool = ctx.enter_context(tc.tile_pool(name="p", bufs=3))
        small = ctx.enter_context(tc.tile_pool(name="small", bufs=2))
        outp = ctx.enter_context(tc.tile_pool(name="outp", bufs=2))
        psA = ctx.enter_context(tc.tile_pool(name="psA", bufs=2, space="PSUM"))
        psB = ctx.enter_context(tc.tile_pool(name="psB", bufs=4, space="PSUM"))

        # ---- weights first so k-projection can start ASAP ----
        wq_sb = [wpool.tile([P, DH], MMD, name=f"wq{c}", tag=f"wq{c}") for c in range(6)]
        wk_sb = [wpool.tile([P, DH], MMD, name=f"wk{c}", tag=f"wk{c}") for c in range(6)]
        wv_sb = [wpool.tile([P, DH], MMD, name=f"wv{c}", tag=f"wv{c}") for c in range(6)]
        wo_sb = [wpool.tile([P, D], MMD, name=f"wo{c}", tag=f"wo{c}") for c in range(3)]
        for c in range(6):
            (nc.sync if c % 2 == 0 else nc.gpsimd).dma_start(
                wk_sb[c][:], wk[ts(c, P), :]
            )
        # small tensors go on the scalar queue to keep the bulk queues clear
        bq_sb = [wpool.tile([P, 1], F32, name=f"bq{t}", tag=f"bq{t}") for t in range(3)]
        bk_sb = [wpool.tile([P, 1], F32, name=f"bk{t}", tag=f"bk{t}") for t in range(3)]
        for t in range(3):
            nc.scalar.dma_start(bq_sb[t][:], bq[ts(t, P), :])
            nc.scalar.dma_start(bk_sb[t][:], bk[ts(t, P), :])
        mv_sb = [wpool.tile([P, 1], F32, name=f"mv{st}", tag=f"mv{st}") for st in range(NKT)]
        for st in range(NKT):
            nc.scalar.dma_start(mv_sb[st][:], mv[ts(st, P), :])
        ones6 = wpool.tile([P, HPC], F32, tag="ones6")
        nc.vector.memset(ones6[:], 1.0)

        # ---- persistent activations ----
        khT = [persist.tile([P, SK], MMD, name=f"khT{t}", tag=f"khT{t}") for t in range(3)]
        qhT = [persist.tile([P, S], MMD, name=f"qhT{t}", tag=f"qhT{t}") for t in range(3)]
        vh = [persist.tile([P, VW], MMD, name=f"vh{st}", tag=f"vh{st}") for st in range(NKT)]
        cn = [persist.tile([P, S], MMD, name=f"cn{t}", tag=f"cn{t}") for t in range(3)]

        # ---- phase 1a: k/v/q projections (q chunks 1.. deferred into the
        # attention stream as PE filler work) ----
        def proj_chunk(xdram, wsb, bsb, dst, off, w):
            xt = [xin.tile([P, 512], MMD, name="xin", tag="xin") for c in range(6)]
            for c in range(6):
                (nc.sync if c % 2 == 0 else nc.gpsimd).dma_start(
                    xt[c][:, :w], xdram[ts(c, P), ds(off, w)]
                )
            for dt in range(3):
                ps = psA.tile([P, 1024], F32, name="psA", tag="psA")
                for c in range(6):
                    nc.tensor.matmul(
                        ps[:, :w],
                        lhsT=wsb[c][:, ts(dt, P)],
                        rhs=xt[c][:, :w],
                        start=(c == 0),
                        stop=(c == 5),
                    )
                nc.vector.tensor_scalar_add(
                    out=dst[dt][:, ds(off, w)], in0=ps[:, :w],
                    scalar1=bsb[dt][:],
                )

        # k-projection
        for off, w in KCH:
            proj_chunk(kT, wk_sb, bk_sb, khT, off, w)

        # v-projection
        for c in range(6):
            nc.sync.dma_start(wv_sb[c][:], wv[ts(c, P), :])
        for off, w in KCH:
            vt = [xin.tile([P, 512], MMD, name="xin", tag="xin") for c in range(6)]
            for c in range(6):
                (nc.sync if c % 2 == 0 else nc.gpsimd).dma_start(
                    vt[c][:, :w], vT[ts(c, P), ds(off, w)]
                )
            for sj in range(w // P):
                st = off // P + sj
                ps = psA.tile([P, 1024], F32, name="psA", tag="psA")
                for c in range(6):
                    nc.tensor.matmul(
                        ps[:, :DH],
                        lhsT=vt[c][:, ts(sj, P)],
                        rhs=wv_sb[c][:],
                        start=(c == 0),
                        stop=(c == 5),
                    )
                vh3 = vh[st].rearrange("p (h c) -> p h c", c=65)
                nc.vector.tensor_scalar_mul(
                    out=vh3[:, :, 0:64],
                    in0=ps[:, :DH].rearrange("p (h c) -> p h c", c=64),
                    scalar1=mv_sb[st][:],
                )
                nc.vector.tensor_scalar_mul(
                    out=vh3[:, :, 64:65],
                    in0=ones6[:].rearrange("p (h c) -> p h c", c=1),
                    scalar1=mv_sb[st][:],
                )

        # q-projection: first q-block now, rest deferred into phase 2
        for c in range(6):
            nc.sync.dma_start(wq_sb[c][:], wq[ts(c, P), :])
        proj_chunk(qT, wq_sb, bq_sb, qhT, 0, QCH[0][1])
        for c in range(3):
            nc.sync.dma_start(wo_sb[c][:], wo[ts(c, P), :])
        pend_qproj = [
            (ch, dt) for ch in range(1, len(QCH)) for dt in range(3)
        ]
        qproj_xt = {}
        for ch in range(1, len(QCH)):
            qproj_xt[ch] = [
                qdef.tile([P, 512], MMD, name="qx", tag="qx") for c in range(6)
            ]
            for c in range(6):
                (nc.sync if c % 2 == 0 else nc.gpsimd).dma_start(
                    qproj_xt[ch][c][:], qT[ts(c, P), ds(QCH[ch][0], 512)]
                )

        # ---- phase 2: attention, head-pair steps ----
        # Each step handles BOTH heads of a pair for one k-chunk: the two
        # scores matmuls live in disjoint PE row groups (base partition 0
        # and 64) so they run concurrently, and share one [128,1024] PSUM
        # tile (head A in cols 0:512, head B in 512:1024) -> one exp per
        # step. Scores run 2 steps ahead of attn@V. Filler work is placed
        # so transient PSUM use never exceeds the free banks: drains right
        # at group start (kc 1,2) free the previous ctx pair early, then
        # q-proj (kc 4,6) and O-proj halves (odd kc>=3) fill PE slack.
        hq = [(pr, qb) for qb in range(NQB) for pr in range(3)]
        steps = [(pr, qb, kc) for (pr, qb) in hq for kc in range(NKT)]
        drain_slots = (1, 2)
        qproj_slots = (4, 6)
        oproj_slots = tuple(k for k in range(3, NKT) if k % 2 == 1)

        ctx_ps = {}
        st_ps = {}

        def scores(pr, qb, kc):
            ps = psA.tile([P, 1024], F32, name="psA", tag="psA")
            for hh in range(2):
                nc.tensor.matmul(
                    ps[:, ts(hh, 512)],
                    lhsT=khT[pr][64 * hh : 64 * hh + 64, ts(kc, P)],
                    rhs=qhT[pr][64 * hh : 64 * hh + 64, ts(qb, QBW)],
                    start=True,
                    stop=True,
                )
            st_ps[(pr, qb, kc)] = ps

        def attnv(pr, qb, kc, pt):
            for hh in range(2):
                h = 2 * pr + hh
                nc.tensor.matmul(
                    ctx_ps[(h, qb)][0:65, :],
                    lhsT=vh[kc][:, ds(65 * h, 65)],
                    rhs=pt[:, ts(hh, 512)],
                    start=(kc == 0),
                    stop=(kc == NKT - 1),
                )

        def drain(h, qb):
            """Normalize + store ctx for a finished (h, qb)."""
            dt, pb = h // 2, 64 * (h % 2)
            cps = ctx_ps.pop((h, qb))
            # reciprocal_approx_fast can't read partition-offset APs (the
            # custom-DVE encoding drops the partition base), so stage the
            # denominator row at partition 0 first.
            dcp = small.tile([1, QBW], F32, name="dcp", tag="dcp")
            nc.vector.tensor_copy(dcp[:], cps[64:65, :])
            rs = small.tile([1, QBW], F32, name="rs", tag="rs")
            nc.vector.reciprocal_approx_fast(rs[:], dcp[:])
            bcs = small.tile([64, QBW], F32, name="bcs", tag="bcs")
            nc.gpsimd.partition_broadcast(bcs[:], rs[:])
            if pb == 0:
                nc.vector.tensor_tensor(
                    out=cn[dt][0:64, ts(qb, QBW)],
                    in0=cps[0:64, :],
                    in1=bcs[:],
                    op=mybir.AluOpType.mult,
                )
            else:
                tmp = small.tile([64, QBW], MMD, name="tmp", tag="tmp")
                nc.vector.tensor_tensor(
                    out=tmp[:], in0=cps[0:64, :], in1=bcs[:],
                    op=mybir.AluOpType.mult,
                )
                nc.sync.dma_start(cn[dt][64:128, ts(qb, QBW)], tmp[:])

        def oproj_half(qc, n0, nw):
            ps = psB.tile([P, 512], F32, name="psB", tag="psB")
            for dt in range(3):
                nc.tensor.matmul(
                    ps[:, :nw],
                    lhsT=cn[dt][:, ts(qc, P)],
                    rhs=wo_sb[dt][:, ds(n0, nw)],
                    start=(dt == 0),
                    stop=(dt == 2),
                )
            ot = outp.tile([P, 512], F32, name="ot", tag="ot")
            nc.vector.tensor_copy(ot[:, :nw], ps[:, :nw])
            nc.sync.dma_start(out[ts(qc, P), ds(n0, nw)], ot[:, :nw])

        def qproj_sub(ch, dt):
            ps = psB.tile([P, 512], F32, name="psB", tag="psB")
            for c in range(6):
                nc.tensor.matmul(
                    ps[:],
                    lhsT=wq_sb[c][:, ts(dt, P)],
                    rhs=qproj_xt[ch][c][:],
                    start=(c == 0),
                    stop=(c == 5),
                )
            nc.vector.tensor_scalar_add(
                out=qhT[dt][:, ds(QCH[ch][0], 512)], in0=ps[:],
                scalar1=bq_sb[dt][:],
            )
            if dt == 2:
                qproj_xt.pop(ch)

        DEPTH = 2
        pend_drain = []
        pend_oproj = []
        for n, (pr, qb, kc) in enumerate(steps):
            if kc == 0:
                for hh in range(2):
                    ctx_ps[(2 * pr + hh, qb)] = psB.tile(
                        [P, QBW], F32, name="psB", tag="psB"
                    )[0:65, :]
            if n < DEPTH:
                scores(*steps[n])
            pt = ppool.tile([P, 1024], MMD, name="pt", tag="pt")
            nc.scalar.activation(
                pt[:], st_ps.pop((pr, qb, kc))[:],
                mybir.ActivationFunctionType.Exp, scale=0.125,
            )
            if n + DEPTH < len(steps):
                scores(*steps[n + DEPTH])
            attnv(pr, qb, kc, pt)
            if kc in drain_slots and pend_drain:
                hd, qd = pend_drain.pop(0)
                drain(hd, qd)
                if hd == HPC - 1:
                    for qc in range(qd * (QBW // P), (qd + 1) * (QBW // P)):
                        pend_oproj.append((qc, 0, 512))
                        pend_oproj.append((qc, 512, 256))
            elif kc in qproj_slots and pend_qproj:
                qproj_sub(*pend_qproj.pop(0))
            elif kc in oproj_slots and pend_oproj:
                oproj_half(*pend_oproj.pop(0))
            if kc == NKT - 1:
                pend_drain.extend([(2 * pr, qb), (2 * pr + 1, qb)])
        for hd, qd in pend_drain:
            drain(hd, qd)
            if hd == HPC - 1:
                for qc in range(qd * (QBW // P), (qd + 1) * (QBW // P)):
                    pend_oproj.append((qc, 0, 512))
                    pend_oproj.append((qc, 512, 256))
        for qc, n0, nw in pend_oproj:
            oproj_half(qc, n0, nw)

    nc.compile()
    return nc


_NC_CACHE = {}


def _get_nc(S, SK, bf16=True):
    key = (S, SK, bf16)
    if key not in _NC_CACHE:
        _NC_CACHE[key] = build_nc(S, SK, bf16)
    return _NC_CACHE[key]


def _install_ntff_hook():
    try:
        mod = types.ModuleType("antenv.axon_hooks")
        state = {"hook": None}
        mod.set_axon_ntff_profile_hook = lambda h: state.__setitem__("hook", h)
        mod.get_axon_ntff_profile_hook = lambda: state["hook"]
        sys.modules["antenv.axon_hooks"] = mod
        from trn_agent_boot.trn_boot import _ntff_profile_via_ctypes

        mod.set_axon_ntff_profile_hook(
            _ntff_profile_via_ctypes("/opt/axon/libaxon_pjrt.so")
        )
        bass_utils.upload_artifacts = lambda tmpdir: "local://" + tmpdir
        return state["hook"] is not None
    except Exception:
        return False


def run_cores(in_maps, S=2048, SK=1152, bf16=True, profile=False):
    nc = _get_nc(S, SK, bf16)
    trace = bool(profile) and _install_ntff_hook()
    res = bass_utils.run_bass_kernel_spmd(
        nc, in_maps, core_ids=list(range(len(in_maps))), trace=trace
    )
    return res


def make_in_maps(q, k, v, mask, Wq, bq, Wk, bk, Wv, Wo, bf16=True):
    B, S, _ = q.shape
    mmd = ml_dtypes.bfloat16 if bf16 else np.float32
    mbool = np.asarray(mask).reshape(B, S)
    keep = [np.nonzero(~mbool[b])[0] for b in range(B)]
    nmax = max((len(ix) for ix in keep), default=1)
    SK = max(128, ((int(nmax) + 127) // 128) * 128)

    qT = np.ascontiguousarray(
        np.asarray(q, np.float32).transpose(0, 2, 1)).astype(mmd)
    kT33 = np.asarray(k, np.float32).transpose(0, 2, 1)
    vT33 = np.asarray(v, np.float32).transpose(0, 2, 1)
    kTc = np.zeros((B, D, SK), np.float32)
    vTc = np.zeros((B, D, SK), np.float32)
    mvec = np.zeros((B, SK), np.float32)
    for b in range(B):
        nb = len(keep[b])
        kTc[b, :, :nb] = kT33[b][:, keep[b]]
        vTc[b, :, :nb] = vT33[b][:, keep[b]]
        mvec[b, :nb] = 1.0
    kTc = kTc.astype(mmd)
    vTc = vTc.astype(mmd)
    Wq, Wk, Wv, Wo = (np.asarray(a, np.float32) for a in (Wq, Wk, Wv, Wo))
    bq, bk = np.asarray(bq, np.float32), np.asarray(bk, np.float32)
    in_maps = []
    for b in range(B):
        for half in range(2):
            hs = slice(DH * half, DH * (half + 1))
            in_maps.append(
                {
                    "qT": qT[b],
                    "kT": kTc[b],
                    "vT": vTc[b],
                    "wq": np.ascontiguousarray(Wq[:, hs]).astype(mmd),
                    "wk": np.ascontiguousarray(Wk[:, hs]).astype(mmd),
                    "wv": np.ascontiguousarray(Wv[:, hs]).astype(mmd),
                    "wo": np.ascontiguousarray(Wo[hs, :]).astype(mmd),
                    "bq": np.ascontiguousarray(bq[hs]).reshape(DH, 1),
                    "bk": np.ascontiguousarray(bk[hs]).reshape(DH, 1),
                    "mv": np.ascontiguousarray(mvec[b]).reshape(-1, 1),
                }
            )
    return in_maps, SK


def kernel(q, k, v, mask, Wq, bq, Wk, bk, Wv, bv, Wo, bo):
    q = np.asarray(q, np.float32)
    B, S, _ = q.shape
    bf16 = os.environ.get("BASS_PRECISE") != "1"
    in_maps, SK = make_in_maps(
        q, k, v, mask, Wq, bq, Wk, bk, Wv, Wo, bf16=bf16
    )
    res = run_cores(
        in_maps, S=S, SK=SK, bf16=bf16,
        profile=os.environ.get("BASS_PROFILE") == "1",
    )
    if os.environ.get("BASS_PROFILE") == "1" and res.exec_time_ns is not None:
        print(f"HW exec time: {res.exec_time_ns} ns")
    cvec = (
        np.asarray(bv, np.float32) @ np.asarray(Wo, np.float32)
        + np.asarray(bo, np.float32)
    )
    out = np.empty((B, S, D), np.float32)
    for b in range(B):
        out[b] = res.results[2 * b]["out"] + res.results[2 * b + 1]["out"] + cvec
    return out


# revision 11
# speedup vs baseline: 1.8772x; 1.0406x over previous
"""MultiHeadAttention TRN2 Bass kernel.

Problem: B=4, S=2048, D=768, H=12 heads (DK=64).
Sharding: 8 cores = (batch b in 0..3) x (head-half in 0..1); each core
computes 6 heads of one batch element end-to-end (tensor-parallel over
heads within a batch). Host pre-transposes activations to [D, S] (and
casts to bf16 in the default fast path), slices projection weights per
head-half, and sums the two partial outputs per batch (+ bv@Wo + bo
correction, exact because softmax rows sum to 1).

Key compaction: the boolean mask drops ~half the key positions, so the
host gathers the unmasked keys to the front of kT/vT (padded with zeros
to a multiple of 128, SK columns total) and sets mv=1 for real keys /
0 for padding. The padded columns flow through the same mask-fold math
(vh_aug *= mv) the full kernel used, so the result is exact while the
score/exp/attn@V/k-proj/v-proj work drops to SK/S of the dense cost.

On-core math:
  qh^T[384, S]: lhsT=Wq tile [Din,dout], rhs=q^T tile [Din,s] (+bq in drain)
  kh^T likewise over SK; vh natural [SK, 390] via lhsT=v^T tile, rhs=Wv:
    vh_aug[s, 65j..65j+64] = [m(s)*vh_head_j(s, :), m(s)]  (mask fold)
  S^T[k, q] = kh_head^T.T @ qh_head^T  (contraction d=64)
  P^T = exp(S^T * 0.125)               (ACT, fused scale, no max-sub)
  ctx_aug^T[0:65, q] += vh_aug_j[kc].T @ P^T[kc]  over k-chunks
    rows 0..63 = unnormalized ctx^T, row 64 = softmax denominator
  rs = recip-approx(denom); bcast via gpsimd partition_broadcast;
  cn = ctx^T * rs   (drains deferred one (h,qb) iteration so the PE
                     pipeline never waits on the reciprocal chain)
  out[q, 768] = sum_dt cn[dt].T @ Wo tiles  (per 128-q chunk, split in
  two PSUM-bank-sized halves so transient PSUM stays at 1 bank)
"""

import os
import sys
import types
from contextlib import ExitStack

import ml_dtypes
import numpy as np

import concourse.bacc as bacc
import concourse.bass as bass
import concourse.mybir as mybir
import concourse.tile as tile
from concourse import bass_utils
from concourse.bass import ts, ds

F32 = mybir.dt.float32
F32R = mybir.dt.float32r
BF16 = mybir.dt.bfloat16

D = 768        # model dim
DH = 384       # per-core head dim (6 heads x 64)
HPC = 6        # heads per core
VW = HPC * 65  # vh_aug free width (390)


def _chunks(total, w=512):
    out = []
    off = 0
    while off < total:
        cw = min(w, total - off)
        out.append((off, cw))
        off += cw
    return out


def build_nc(S=2048, SK=1152, bf16=True):
    nc = bacc.Bacc("TRN2", target_bir_lowering=False, debug=False)

    MMD = BF16 if bf16 else F32R    # matmul operand dtype
    QBW = min(512, S)               # attention q-block width
    NKT = SK // 128                 # 128-wide k tiles
    NQB = S // QBW                  # q blocks
    KCH = _chunks(SK)               # k/v projection chunks (<=512 wide)
    QCH = _chunks(S)                # q projection chunks (512 wide)

    qT = nc.dram_tensor("qT", [D, S], MMD, kind="ExternalInput").ap()
    kT = nc.dram_tensor("kT", [D, SK], MMD, kind="ExternalInput").ap()
    vT = nc.dram_tensor("vT", [D, SK], MMD, kind="ExternalInput").ap()
    wq = nc.dram_tensor("wq", [D, DH], MMD, kind="ExternalInput").ap()
    wk = nc.dram_tensor("wk", [D, DH], MMD, kind="ExternalInput").ap()
    wv = nc.dram_tensor("wv", [D, DH], MMD, kind="ExternalInput").ap()
    wo = nc.dram_tensor("wo", [DH, D], MMD, kind="ExternalInput").ap()
    bq = nc.dram_tensor("bq", [DH, 1], F32, kind="ExternalInput").ap()
    bk = nc.dram_tensor("bk", [DH, 1], F32, kind="ExternalInput").ap()
    mv = nc.dram_tensor("mv", [SK, 1], F32, kind="ExternalInput").ap()
    out = nc.dram_tensor("out", [S, D], F32, kind="ExternalOutput").ap()

    with tile.TileContext(nc) as tc, ExitStack() as ctx:
        P = 128
        wpool = ctx.enter_context(tc.tile_pool(name="w", bufs=1))
        xin = ctx.enter_context(tc.tile_pool(name="xin", bufs=12))
        qdef = ctx.enter_context(tc.tile_pool(name="qdef", bufs=18))
        persist = ctx.enter_context(tc.tile_pool(name="persist", bufs=1))
        ppool = ctx.enter_context(tc.tile_pool(name="p", bufs=3))
        small = ctx.enter_context(tc.tile_pool(name="small", bufs=2))
        outp = ctx.enter_context(tc.tile_pool(name="outp", bufs=2))
        psA = ctx.enter_context(tc.tile_pool(name="psA", bufs=2, space="PSUM"))
        psB = ctx.enter_context(tc.tile_pool(name="psB", bufs=4, space="PSUM"))

        # ---- weights first so k-projection can start ASAP ----
        wq_sb = [wpool.tile([P, DH], MMD, name=f"wq{c}", tag=f"wq{c}") for c in range(6)]
        wk_sb = [wpool.tile([P, DH], MMD, name=f"wk{c}", tag=f"wk{c}") for c in range(6)]
        wv_sb = [wpool.tile([P, DH], MMD, name=f"wv{c}", tag=f"wv{c}") for c in range(6)]
        wo_sb = [wpool.tile([P, D], MMD, name=f"wo{c}", tag=f"wo{c}") for c in range(3)]
        # first k-chunk inputs interleave with wk so the first matmul's
        # operands (wk0 + xt0) land as early as possible
        xt0 = [xin.tile([P, 512], MMD, name="xin", tag="xin") for c in range(6)]
        for c in range(6):
            eng = nc.sync if c % 2 == 0 else nc.gpsimd
            eng.dma_start(wk_sb[c][:], wk[ts(c, P), :])
            eng.dma_start(xt0[c][:, : KCH[0][1]], kT[ts(c, P), ds(0, KCH[0][1])])
        # small tensors go on the scalar queue to keep the bulk queues clear
        bq_sb = [wpool.tile([P, 1], F32, name=f"bq{t}", tag=f"bq{t}") for t in range(3)]
        bk_sb = [wpool.tile([P, 1], F32, name=f"bk{t}", tag=f"bk{t}") for t in range(3)]
        for t in range(3):
            nc.scalar.dma_start(bq_sb[t][:], bq[ts(t, P), :])
            nc.scalar.dma_start(bk_sb[t][:], bk[ts(t, P), :])
        mv_sb = [wpool.tile([P, 1], F32, name=f"mv{st}", tag=f"mv{st}") for st in range(NKT)]
        for st in range(NKT):
            nc.scalar.dma_start(mv_sb[st][:], mv[ts(st, P), :])
        ones6 = wpool.tile([P, HPC], F32, tag="ones6")
        nc.vector.memset(ones6[:], 1.0)

        # ---- persistent activations ----
        khT = [persist.tile([P, SK], MMD, name=f"khT{t}", tag=f"khT{t}") for t in range(3)]
        qhT = [persist.tile([P, S], MMD, name=f"qhT{t}", tag=f"qhT{t}") for t in range(3)]
        vh = [persist.tile([P, VW], MMD, name=f"vh{st}", tag=f"vh{st}") for st in range(NKT)]
        cn = [persist.tile([P, S], MMD, name=f"cn{t}", tag=f"cn{t}") for t in range(3)]

        # ---- phase 1a: k/v/q projections (q chunks 1.. deferred into the
        # attention stream as PE filler work) ----
        def proj_chunk(xdram, wsb, bsb, dst, off, w, xt=None):
            # dt-interleaved accumulation: the three dt chains land in three
            # separate PSUM banks so consecutive matmuls never RMW the same
            # bank and the PE pipeline stays full
            if xt is None:
                xt = [xin.tile([P, 512], MMD, name="xin", tag="xin") for c in range(6)]
                for c in range(6):
                    (nc.sync if c % 2 == 0 else nc.gpsimd).dma_start(
                        xt[c][:, :w], xdram[ts(c, P), ds(off, w)]
                    )
            ps3 = [psB.tile([P, 512], F32, name="psB", tag="psB") for _ in range(3)]
            for c in range(6):
                for dt in range(3):
                    nc.tensor.matmul(
                        ps3[dt][:, :w],
                        lhsT=wsb[c][:, ts(dt, P)],
                        rhs=xt[c][:, :w],
                        start=(c == 0),
                        stop=(c == 5),
                    )
            for dt in range(3):
                nc.vector.tensor_scalar_add(
                    out=dst[dt][:, ds(off, w)], in0=ps3[dt][:, :w],
                    scalar1=bsb[dt][:],
                )

        # k-projection (chunk 0's inputs were DMA'd with the weights above)
        for ci, (off, w) in enumerate(KCH):
            proj_chunk(kT, wk_sb, bk_sb, khT, off, w, xt=xt0 if ci == 0 else None)

        # v-projection: st chains pair-interleaved across two PSUM banks
        for c in range(6):
            (nc.sync if c % 2 == 0 else nc.gpsimd).dma_start(
                wv_sb[c][:], wv[ts(c, P), :]
            )

        def vproj_sts(vt, sjs):
            pss = [psB.tile([P, 512], F32, name="psB", tag="psB") for _ in sjs]
            for c in range(6):
                for i, sj in enumerate(sjs):
                    nc.tensor.matmul(
                        pss[i][:, :DH],
                        lhsT=vt[c][:, ts(sj, P)],
                        rhs=wv_sb[c][:],
                        start=(c == 0),
                        stop=(c == 5),
                    )
            for i, sj in enumerate(sjs):
                st = base_st + sj
                vh3 = vh[st].rearrange("p (h c) -> p h c", c=65)
                nc.vector.tensor_scalar_mul(
                    out=vh3[:, :, 0:64],
                    in0=pss[i][:, :DH].rearrange("p (h c) -> p h c", c=64),
                    scalar1=mv_sb[st][:],
                )
                nc.vector.tensor_scalar_mul(
                    out=vh3[:, :, 64:65],
                    in0=ones6[:].rearrange("p (h c) -> p h c", c=1),
                    scalar1=mv_sb[st][:],
                )

        for off, w in KCH:
            vt = [xin.tile([P, 512], MMD, name="xin", tag="xin") for c in range(6)]
            for c in range(6):
                (nc.sync if c % 2 == 0 else nc.gpsimd).dma_start(
                    vt[c][:, :w], vT[ts(c, P), ds(off, w)]
                )
            base_st = off // P
            nst = w // P
            for s0 in range(0, nst, 2):
                vproj_sts(vt, list(range(s0, min(s0 + 2, nst))))

        # q-projection: first q-block now, rest deferred into phase 2
        for c in range(6):
            (nc.sync if c % 2 == 0 else nc.gpsimd).dma_start(
                wq_sb[c][:], wq[ts(c, P), :]
            )
        proj_chunk(qT, wq_sb, bq_sb, qhT, 0, QCH[0][1])
        # deferred q chunks / wo are DMA'd lazily inside the attention loop
        # to keep phase-1 SBUF write bandwidth for the k/v/q-0 streams
        pend_qproj = [
            (ch, dt) for ch in range(1, len(QCH)) for dt in range(3)
        ]
        qproj_xt = {}

        def issue_qdef_dma(ch):
            qproj_xt[ch] = [
                qdef.tile([P, 512], MMD, name="qx", tag="qx") for c in range(6)
            ]
            for c in range(6):
                (nc.sync if c % 2 == 0 else nc.gpsimd).dma_start(
                    qproj_xt[ch][c][:], qT[ts(c, P), ds(QCH[ch][0], 512)]
                )

        # ---- phase 2: attention, head-pair steps ----
        # Each step handles BOTH heads of a pair for one k-chunk: the two
        # scores matmuls live in disjoint PE row groups (base partition 0
        # and 64) so they run concurrently, and share one [128,1024] PSUM
        # tile (head A in cols 0:512, head B in 512:1024) -> one exp per
        # step. Scores run 2 steps ahead of attn@V. Filler work is placed
        # so transient PSUM use never exceeds the free banks: drains right
        # at group start (kc 1,2) free the previous ctx pair early, then
        # q-proj (kc 4,6) and O-proj halves (odd kc>=3) fill PE slack.
        hq = [(pr, qb) for qb in range(NQB) for pr in range(3)]
        steps = [(pr, qb, kc) for (pr, qb) in hq for kc in range(NKT)]
        drain_slots = (1, 2)
        qproj_slots = (4, 6)
        oproj_slots = tuple(k for k in range(3, NKT) if k % 2 == 1)

        ctx_ps = {}
        st_ps = {}

        def scores(pr, qb, kc):
            ps = psA.tile([P, 1024], F32, name="psA", tag="psA")
            for hh in range(2):
                nc.tensor.matmul(
                    ps[:, ts(hh, 512)],
                    lhsT=khT[pr][64 * hh : 64 * hh + 64, ts(kc, P)],
                    rhs=qhT[pr][64 * hh : 64 * hh + 64, ts(qb, QBW)],
                    start=True,
                    stop=True,
                )
            st_ps[(pr, qb, kc)] = ps

        def attnv(pr, qb, kc, pt):
            for hh in range(2):
                h = 2 * pr + hh
                nc.tensor.matmul(
                    ctx_ps[(h, qb)][0:65, :],
                    lhsT=vh[kc][:, ds(65 * h, 65)],
                    rhs=pt[:, ts(hh, 512)],
                    start=(kc == 0),
                    stop=(kc == NKT - 1),
                )

        def drain(h, qb):
            """Normalize + store ctx for a finished (h, qb)."""
            dt, pb = h // 2, 64 * (h % 2)
            cps = ctx_ps.pop((h, qb))
            # reciprocal_approx_fast can't read partition-offset APs (the
            # custom-DVE encoding drops the partition base), so stage the
            # denominator row at partition 0 first.
            dcp = small.tile([1, QBW], F32, name="dcp", tag="dcp")
            nc.vector.tensor_copy(dcp[:], cps[64:65, :])
            rs = small.tile([1, QBW], F32, name="rs", tag="rs")
            nc.vector.reciprocal_approx_fast(rs[:], dcp[:])
            bcs = small.tile([64, QBW], F32, name="bcs", tag="bcs")
            nc.gpsimd.partition_broadcast(bcs[:], rs[:])
            if pb == 0:
                nc.vector.tensor_tensor(
                    out=cn[dt][0:64, ts(qb, QBW)],
                    in0=cps[0:64, :],
                    in1=bcs[:],
                    op=mybir.AluOpType.mult,
                )
            else:
                tmp = small.tile([64, QBW], MMD, name="tmp", tag="tmp")
                nc.vector.tensor_tensor(
                    out=tmp[:], in0=cps[0:64, :], in1=bcs[:],
                    op=mybir.AluOpType.mult,
                )
                nc.sync.dma_start(cn[dt][64:128, ts(qb, QBW)], tmp[:])

        def oproj_half(qc, n0, nw, pool=None):
            if pool is None:
                ps = psB.tile([P, 512], F32, name="psB", tag="psB")
            else:
                ps = pool.tile([P, 1024], F32, name="psA", tag="psA")
            for dt in range(3):
                nc.tensor.matmul(
                    ps[:, :nw],
                    lhsT=cn[dt][:, ts(qc, P)],
                    rhs=wo_sb[dt][:, ds(n0, nw)],
                    start=(dt == 0),
                    stop=(dt == 2),
                )
            ot = outp.tile([P, 512], F32, name="ot", tag="ot")
            nc.vector.tensor_copy(ot[:, :nw], ps[:, :nw])
            nc.sync.dma_start(out[ts(qc, P), ds(n0, nw)], ot[:, :nw])

        def qproj_sub(ch, dt):
            ps = psB.tile([P, 512], F32, name="psB", tag="psB")
            for c in range(6):
                nc.tensor.matmul(
                    ps[:],
                    lhsT=wq_sb[c][:, ts(dt, P)],
                    rhs=qproj_xt[ch][c][:],
                    start=(c == 0),
                    stop=(c == 5),
                )
            nc.vector.tensor_scalar_add(
                out=qhT[dt][:, ds(QCH[ch][0], 512)], in0=ps[:],
                scalar1=bq_sb[dt][:],
            )
            if dt == 2:
                qproj_xt.pop(ch)

        DEPTH = 2
        pend_drain = []
        pend_oproj = []
        for n, (pr, qb, kc) in enumerate(steps):
            grp = n // NKT
            if kc == 0:
                # lazy bulk DMAs: deferred q chunks at groups 0..; wo at 1
                if grp + 1 < len(QCH):
                    issue_qdef_dma(grp + 1)
                if grp == 1:
                    for c in range(3):
                        nc.sync.dma_start(wo_sb[c][:], wo[ts(c, P), :])
                for hh in range(2):
                    ctx_ps[(2 * pr + hh, qb)] = psB.tile(
                        [P, QBW], F32, name="psB", tag="psB"
                    )[0:65, :]
            if n < DEPTH:
                scores(*steps[n])
            pt = ppool.tile([P, 1024], MMD, name="pt", tag="pt")
            nc.scalar.activation(
                pt[:], st_ps.pop((pr, qb, kc))[:],
                mybir.ActivationFunctionType.Exp, scale=0.125,
            )
            if n + DEPTH < len(steps):
                scores(*steps[n + DEPTH])
            attnv(pr, qb, kc, pt)
            if kc in drain_slots and pend_drain:
                hd, qd = pend_drain.pop(0)
                drain(hd, qd)
                if hd == HPC - 1:
                    for qc in range(qd * (QBW // P), (qd + 1) * (QBW // P)):
                        pend_oproj.append((qc, 0, 512))
                        pend_oproj.append((qc, 512, 256))
            elif kc in qproj_slots and pend_qproj and grp >= 1:
                qproj_sub(*pend_qproj.pop(0))
            elif kc in oproj_slots and pend_oproj:
                oproj_half(*pend_oproj.pop(0))
            if kc == NKT - 1:
                pend_drain.extend([(2 * pr, qb), (2 * pr + 1, qb)])
        for hd, qd in pend_drain:
            drain(hd, qd)
            if hd == HPC - 1:
                for qc in range(qd * (QBW // P), (qd + 1) * (QBW // P)):
                    pend_oproj.append((qc, 0, 512))
                    pend_oproj.append((qc, 512, 256))
        # tail: alternate psA (idle now) and psB so up to 6 halves in flight
        for i, (qc, n0, nw) in enumerate(pend_oproj):
            oproj_half(qc, n0, nw, pool=psA if i % 2 == 0 else None)

    nc.compile()
    return nc


_NC_CACHE = {}


def _get_nc(S, SK, bf16=True):
    key = (S, SK, bf16)
    if key not in _NC_CACHE:
        _NC_CACHE[key] = build_nc(S, SK, bf16)
    return _NC_CACHE[key]


def _install_ntff_hook():
    try:
        mod = types.ModuleType("antenv.axon_hooks")
        state = {"hook": None}
        mod.set_axon_ntff_profile_hook = lambda h: state.__setitem__("hook", h)
        mod.get_axon_ntff_profile_hook = lambda: state["hook"]
        sys.modules["antenv.axon_hooks"] = mod
        from trn_agent_boot.trn_boot import _ntff_profile_via_ctypes

        mod.set_axon_ntff_profile_hook(
            _ntff_profile_via_ctypes("/opt/axon/libaxon_pjrt.so")
        )
        bass_utils.upload_artifacts = lambda tmpdir: "local://" + tmpdir
        return state["hook"] is not None
    except Exception:
        return False


def run_cores(in_maps, S=2048, SK=1152, bf16=True, profile=False):
    nc = _get_nc(S, SK, bf16)
    trace = bool(profile) and _install_ntff_hook()
    res = bass_utils.run_bass_kernel_spmd(
        nc, in_maps, core_ids=list(range(len(in_maps))), trace=trace
    )
    return res


def make_in_maps(q, k, v, mask, Wq, bq, Wk, bk, Wv, Wo, bf16=True):
    B, S, _ = q.shape
    mmd = ml_dtypes.bfloat16 if bf16 else np.float32
    mbool = np.asarray(mask).reshape(B, S)
    keep = [np.nonzero(~mbool[b])[0] for b in range(B)]
    nmax = max((len(ix) for ix in keep), default=1)
    SK = max(128, ((int(nmax) + 127) // 128) * 128)

    qT = np.ascontiguousarray(
        np.asarray(q, np.float32).transpose(0, 2, 1)).astype(mmd)
    kT33 = np.asarray(k, np.float32).transpose(0, 2, 1)
    vT33 = np.asarray(v, np.float32).transpose(0, 2, 1)
    kTc = np.zeros((B, D, SK), np.float32)
    vTc = np.zeros((B, D, SK), np.float32)
    mvec = np.zeros((B, SK), np.float32)
    for b in range(B):
        nb = len(keep[b])
        kTc[b, :, :nb] = kT33[b][:, keep[b]]
        vTc[b, :, :nb] = vT33[b][:, keep[b]]
        mvec[b, :nb] = 1.0
    kTc = kTc.astype(mmd)
    vTc = vTc.astype(mmd)
    Wq, Wk, Wv, Wo = (np.asarray(a, np.float32) for a in (Wq, Wk, Wv, Wo))
    bq, bk = np.asarray(bq, np.float32), np.asarray(bk, np.float32)
    in_maps = []
    for b in range(B):
        for half in range(2):
            hs = slice(DH * half, DH * (half + 1))
            in_maps.append(
                {
                    "qT": qT[b],
                    "kT": kTc[b],
                    "vT": vTc[b],
                    "wq": np.ascontiguousarray(Wq[:, hs]).astype(mmd),
                    "wk": np.ascontiguousarray(Wk[:, hs]).astype(mmd),
                    "wv": np.ascontiguousarray(Wv[:, hs]).astype(mmd),
                    "wo": np.ascontiguousarray(Wo[hs, :]).astype(mmd),
                    "bq": np.ascontiguousarray(bq[hs]).reshape(DH, 1),
                    "bk": np.ascontiguousarray(bk[hs]).reshape(DH, 1),
                    "mv": np.ascontiguousarray(mvec[b]).reshape(-1, 1),
                }
            )
    return in_maps, SK


def kernel(q, k, v, mask, Wq, bq, Wk, bk, Wv, bv, Wo, bo):
    q = np.asarray(q, np.float32)
    B, S, _ = q.shape
    bf16 = os.environ.get("BASS_PRECISE") != "1"
    in_maps, SK = make_in_maps(
        q, k, v, mask, Wq, bq, Wk, bk, Wv, Wo, bf16=bf16
    )
    res = run_cores(
        in_maps, S=S, SK=SK, bf16=bf16,
        profile=os.environ.get("BASS_PROFILE") == "1",
    )
    if os.environ.get("BASS_PROFILE") == "1" and res.exec_time_ns is not None:
        print(f"HW exec time: {res.exec_time_ns} ns")
    cvec = (
        np.asarray(bv, np.float32) @ np.asarray(Wo, np.float32)
        + np.asarray(bo, np.float32)
    )
    out = np.empty((B, S, D), np.float32)
    for b in range(B):
        out[b] = res.results[2 * b]["out"] + res.results[2 * b + 1]["out"] + cvec
    return out
